# revision 81
# baseline (speedup 1.0000x reference)
"""TRN2 Bass kernel for nn_Attention_90460601189287.

Causal multi-head attention (B=2, N=2048, D=1024, H=16) with spectral-norm
(power-iteration) scaled qkv/proj dense layers, on 8 NeuronCores.

Sharding: tensor-parallel over heads. Core c owns heads {2c, 2c+1}: it gets
the matching 128 columns of each of W_qkv's q/k/v blocks and the matching
128 rows of W_proj, computes attention for its heads over the full batch,
and produces a partial y = x_att @ W_proj_rows. The host sums the 8
partials (the gather step for row-sharded matmul).

All device matmul inputs are bf16 (1 cyc/row on PE at any tile size).
Host-side prep removes every PE transpose and on-device scale:
  - x is transposed on host (xt param [D, NTOK] bf16) so qkv^T comes from
    natural-layout W as lhsT with xt as rhs, and V in natural [tok, hd]
    layout comes from xt-slices as lhsT with Wv as rhs.
  - The spectral-norm scales (computed on host in fp32, identical math to
    the reference) plus the hd^-0.5 attention scale are folded into the
    bf16 weights: Wq,Wk *= c_qkv*hd^-0.25, Wv *= c_qkv, Wp *= c_proj.

Attention per (window g of 512 q-tokens, head h, k-block kb of 128):
  S^T[k,q] = K Q^T in PSUM f32; exp(S - 30) on ACT straight to bf16
  (constant shift replaces the row-max pass; exact after normalization);
  causal mask multiply on diagonal blocks (DVE, bf16 2x mode);
  PV reoriented: out[q, hd|den] = a_t-slice^T @ [V | ones] accumulated in
  PSUM per 128-q-subblock (free size 65 per matmul, and the ones column
  yields the softmax denominator per q-PARTITION, so normalization is a
  per-partition scalar multiply); small PE transpose brings x_att back to
  [d, tok] layout for the proj matmuls.

Emission is software-pipelined: stage A(w+1) (qkv+V matmuls) and stage
C(w-1) (x_att transpose + proj + y writeback) are interleaved into stage
B(w)'s S->exp->PV bubbles so the PE stream stays dense (keeps the PE
p-state at 2.4 GHz and hides the exp latency).
"""
from contextlib import ExitStack

import numpy as np
import ml_dtypes

import concourse.bass as bass
import concourse.mybir as mybir
from concourse.bass_utils import run_bass_kernel_spmd
from concourse.masks import make_identity
from concourse.tile import TileContext

F32 = mybir.dt.float32
F32R = mybir.dt.float32r
BF16 = mybir.dt.bfloat16

N_CORES = 8
BATCH = 2
NTOK = 4096      # flattened b*n
D = 1024
NH = 2           # heads per core
HD = 64
B = 2
NSEQ = 2048
WQ = 512         # token window
NW = NTOK // WQ  # 8 windows
NWB = NSEQ // WQ  # 4 windows per batch
KB = 128
SHIFT = 30.0


# ---------------------------------------------------------------------------
# Workaround: this walrus build accepts at most ONE sync wait per
# instruction. Hoist extra waits onto single-wait NOPs inserted before.
# ---------------------------------------------------------------------------
def _split_sync_waits(nc, max_waits=1):
    for f in nc.m.functions:
        for blk in f.blocks:
            insts = blk.instructions
            out = []
            changed = False
            for inst in insts:
                si = inst.sync_info
                waits = list(si.on_wait) if si is not None else []
                if len(waits) > max_waits:
                    extra = waits[:-max_waits]
                    for i in range(0, len(extra), max_waits):
                        nop = mybir.InstNoOp(name=f"I-{nc.next_id()}", ins=[],
                                             outs=[], engine=inst.engine)
                        nop.sync_info = mybir.SyncInfo(
                            on_wait=extra[i:i + max_waits], on_update=[])
                        nc.register_instruction(nop, overwrite=True)
                        out.append(nop)
                    si.on_wait = waits[-max_waits:]
                    inst.sync_info = si
                    changed = True
                out.append(inst)
            if changed:
                blk.instructions = out


class _TileContextSplit(TileContext):
    def __exit__(self, exc_type, exc_value, traceback):
        ret = super().__exit__(exc_type, exc_value, traceback)
        if exc_type is None:
            _split_sync_waits(self.nc)
        return ret


def declare_params(nc):
    xt = nc.declare_dram_parameter("xt", [D, NTOK], BF16, isOutput=False)
    wqkv = nc.declare_dram_parameter("wqkv", [D, 3 * NH * HD], BF16,
                                     isOutput=False)
    wp = nc.declare_dram_parameter("wp", [NH * HD, D], BF16, isOutput=False)
    mask = nc.declare_dram_parameter("mask", [128, 896], BF16, isOutput=False)
    y = nc.declare_dram_parameter("y", [NTOK, D], BF16, isOutput=True)
    return xt, wqkv, wp, mask, y


def _build_body(nc, tc):
    EXP = mybir.ActivationFunctionType.Exp
    xt, wqkv, wp, mask, y = declare_params(nc)

    ctx = ExitStack()
    with ctx:
        singles = ctx.enter_context(tc.tile_pool(name="singles", bufs=1))

        # ---- static SBUF state ----
        mask_sb = singles.tile([128, 896], BF16)
        shift_sb = singles.tile([128, 1], F32)
        nc.gpsimd.memset(shift_sb[:], -SHIFT)

        ident_f = singles.tile([128, 128], F32)
        make_identity(nc, ident_f)
        ident_r = singles.tile([128, 128], F32)
        nc.vector.tensor_copy(ident_r[:].bitcast(F32R), ident_f[:])

        wqkv_sb = singles.tile([128, 8, 3 * 128], BF16)
        wp_sb = singles.tile([128, D], BF16)

        # per-window persistent tensors (bf16)
        qT = [singles.tile([128, WQ], BF16, name=f"qT_{w}") for w in range(NW)]
        kT = [singles.tile([128, WQ], BF16, name=f"kT_{w}") for w in range(NW)]
        xaw = [singles.tile([128, WQ], BF16, name=f"xa_{w}") for w in range(NW)]
        # V natural layout + ones column, per (head, window, k-subblock):
        # [128 k-tok, slot, kb%4, 64 v | 1 one]
        vnat = singles.tile([128, NH * NW, 4, HD + 1], BF16, name="vnat")
        nc.gpsimd.memset(vnat[:, :, :, HD:HD + 1], 1.0)
        # zeros row: matmul "bank opener". start=True on ANY matmul zeroes
        # the whole target PSUM bank, so banks holding several interleaved
        # accumulation groups are zeroed ONCE via this outer product and
        # every real matmul then runs with start=False.
        zrow = singles.tile([1, 512], BF16, name="zrow")
        nc.gpsimd.memset(zrow[:], 0.0)

        def open_bank(out_ap, n):
            nc.tensor.matmul(out_ap, zrow[0:1, 0:128], zrow[0:1, 0:n],
                             start=True, stop=True, skip_group_check=True)

        # ---- pools ----
        xw_pool = ctx.enter_context(tc.tile_pool(name="xw", bufs=4))
        a_pool = ctx.enter_context(tc.tile_pool(name="apool", bufs=4))
        xan_pool = ctx.enter_context(tc.tile_pool(name="xan", bufs=6))
        rden_pool = ctx.enter_context(tc.tile_pool(name="rden", bufs=4))
        y_pool = ctx.enter_context(tc.tile_pool(name="ypool", bufs=4))
        # PSUM: q1 + k1 + vn1 + s3 + o2 = 8 banks
        ps = ctx.enter_context(tc.tile_pool(name="ps", bufs=1, space="PSUM"))

        xw_tiles = {}

        def mk_xw_dma(w, split=False):
            def op():
                xw_t = xw_pool.tile([128, 8, WQ], BF16, tag="xw", name="xw_t")
                src = xt[:, w * WQ:(w + 1) * WQ].rearrange(
                    "(c p) t -> p c t", p=128)
                if split:  # interleave weight/x quarters so A(0)'s
                    # first dm-chunks start as early as possible
                    wsrc = wqkv.rearrange("(c p) m -> p c m", p=128)
                    for qt in range(4):
                        qs = slice(2 * qt, 2 * qt + 2)
                        nc.sync.dma_start(out=wqkv_sb[:, qs, :],
                                          in_=wsrc[:, qs, :])
                        nc.sync.dma_start(out=xw_t[:, qs, :],
                                          in_=src[:, qs, :])
                    nc.sync.dma_start(out=wp_sb[:], in_=wp[:])
                    nc.sync.dma_start(out=mask_sb[:], in_=mask[:])
                else:
                    nc.sync.dma_start(out=xw_t[:], in_=src)
                xw_tiles[w] = xw_t
            return op

        # ---- stage A: qkv^T (q,k) + V natural for one token window.
        # Two half-window passes so q+k share a single PSUM bank. ----
        def stage_a_ops(w):
            state = {}

            def start():
                state["vn"] = ps.tile([128, 4, 128], F32, tag="vn",
                                      name="vn_ps")
                open_bank(state["vn"][:], 512)

            def mk_dm(p, dm):
                ts = slice(p * 256, (p + 1) * 256)

                def op():
                    if dm == 0:
                        state["qk"] = ps.tile([128, 2, 256], F32, tag="qk",
                                              name="qk_ps")
                        open_bank(state["qk"][:], 512)
                    xw_t = xw_tiles[w]
                    qk = state["qk"]
                    nc.tensor.matmul(qk[:, 0, :], wqkv_sb[:, dm, 0:128],
                                     xw_t[:, dm, ts],
                                     start=False, stop=(dm == 7),
                                     skip_group_check=True)
                    nc.tensor.matmul(qk[:, 1, :], wqkv_sb[:, dm, 128:256],
                                     xw_t[:, dm, ts],
                                     start=False, stop=(dm == 7),
                                     skip_group_check=True)
                    for j in range(2 * p, 2 * p + 2):
                        nc.tensor.matmul(
                            state["vn"][:, j, :],
                            xw_t[:, dm, j * 128:(j + 1) * 128],
                            wqkv_sb[:, dm, 256:384],
                            start=False, stop=(dm == 7),
                            skip_group_check=True)
                return op

            def mk_fin(p):
                ts = slice(p * 256, (p + 1) * 256)

                def op():
                    qk = state["qk"]
                    nc.vector.tensor_copy(qT[w][:, ts], qk[:, 0, :])
                    nc.vector.tensor_copy(kT[w][:, ts], qk[:, 1, :])
                    if p == 1:
                        for h in range(NH):
                            nc.vector.tensor_copy(
                                vnat[:, w * NH + h, :, 0:HD],
                                state["vn"][:, :, h * HD:(h + 1) * HD])
                        del xw_tiles[w]
                return op

            ops = [start]
            for p in range(2):
                ops += [mk_dm(p, dm) for dm in range(8)]
                ops.append(mk_fin(p))
            return ops

        # ---- stage B: attention for one (batch, q-window) ----
        def stage_b(w, filler, final=False):
            b, g = divmod(w, NWB)
            nkb = 4 * (g + 1)
            o_ps = [ps.tile([128, 4, HD + 1], F32, tag="o", bufs=2,
                            name=f"o_ps{h}") for h in range(NH)]
            for h in range(NH):
                open_bank(o_ps[h][:], 4 * (HD + 1))
            fst = {}
            pend_wb = []
            fi = iter(filler)
            nf = len(filler)
            emitted = 0
            for kb in range(nkb):
                kw = b * NWB + kb // 4
                ko = (kb % 4) * KB
                sq = max(0, (kb - 4 * g) * KB)
                diag = kb >= 4 * g
                a_t = a_pool.tile([128, NH, WQ], BF16, tag="a", name="a_t")
                for h in range(NH):
                    hs = slice(h * HD, (h + 1) * HD)
                    s_t = ps.tile([128, WQ], F32, tag="s", bufs=2,
                                  name="s_ps")
                    nc.tensor.matmul(s_t[:, sq:WQ],
                                     kT[kw][hs, ko:ko + KB],
                                     qT[w][hs, sq:WQ],
                                     start=True, stop=True)
                    nc.scalar.activation(a_t[:, h, sq:WQ], s_t[:, sq:WQ],
                                         EXP, bias=shift_sb[:, 0:1],
                                         scale=1.0)
                    if diag:
                        nc.vector.tensor_tensor(
                            out=a_t[:, h, sq:WQ], in0=a_t[:, h, sq:WQ],
                            in1=mask_sb[:, 384:896 - sq],
                            op=mybir.AluOpType.mult)
                # spread filler (prev-window proj, next-window qkv) into the
                # exp-latency bubble between S and PV
                want = int(((kb + 1) / nkb)
                           ** (0.75 if nkb <= 8 else 1.0) * nf)
                while emitted < want:
                    next(fi)()
                    emitted += 1
                for h in range(NH):
                    for jq in range(max(0, kb - 4 * g), 4):
                        nc.tensor.matmul(
                            o_ps[h][:, jq, :],
                            a_t[:, h, jq * 128:(jq + 1) * 128],
                            vnat[:, kw * NH + h, kb % 4, :],
                            start=False, stop=(kb == 4 * g + jq),
                            skip_group_check=True)
                if kb >= 4 * g:
                    # per-q-subblock norm as soon as its PV accumulation
                    # stopped; on the last window the transpose->proj->
                    # writeback chain follows, deferred one round so the
                    # DVE norm latency hides behind the next round's S work
                    jq = kb - 4 * g
                    if jq == 0:
                        fst["rden"] = [rden_pool.tile([128, 4], F32, tag="rd",
                                                      name="rden")
                                       for _ in range(NH)]
                        fst["xan"] = xan_pool.tile([128, 4, 2 * HD], F32,
                                                   tag="xan", name="xan")
                        if final:
                            fst["xaT"] = ps.tile([128, WQ], F32, tag="vn",
                                                 bufs=1, name="xaT_ps")
                    for h in range(NH):
                        hs = slice(h * HD, (h + 1) * HD)
                        nc.vector.reciprocal(fst["rden"][h][:, jq:jq + 1],
                                             o_ps[h][:, jq, HD:HD + 1])
                        nc.vector.tensor_scalar_mul(
                            fst["xan"][:, jq, hs].bitcast(F32R),
                            o_ps[h][:, jq, 0:HD],
                            fst["rden"][h][:, jq:jq + 1])

                    def mk_wb(jq):
                        def wb():
                            js = slice(jq * 128, (jq + 1) * 128)
                            nc.tensor.transpose(
                                fst["xaT"][:, js].bitcast(F32R),
                                fst["xan"][:, jq, :].bitcast(F32R),
                                ident_r[:].bitcast(F32R))
                            nc.vector.tensor_copy(xaw[w][:, js],
                                                  fst["xaT"][:, js])
                            nb = 4 * w + jq
                            y_sb = y_pool.tile([128, D], BF16, tag="y",
                                               name="y_sb")
                            for cc in range(2):
                                yp = ps.tile([128, 512], F32, tag="yx",
                                             bufs=2, name="yp")
                                nc.tensor.matmul(
                                    yp[:], xaw[w][:, js],
                                    wp_sb[:, cc * 512:(cc + 1) * 512],
                                    start=True, stop=True)
                                _copy(("vector", "scalar")[cc],
                                      y_sb[:, cc * 512:(cc + 1) * 512],
                                      yp[:])
                                if jq == 3:  # last block: fly each half as
                                    # soon as its copy lands (shorter tail)
                                    nc.sync.dma_start(
                                        out=y[nb * 128:(nb + 1) * 128,
                                              cc * 512:(cc + 1) * 512],
                                        in_=y_sb[:, cc * 512:(cc + 1) * 512])
                            if jq != 3:
                                nc.sync.dma_start(
                                    out=y[nb * 128:(nb + 1) * 128, :],
                                    in_=y_sb[:])
                        return wb
                    if final:
                        pend_wb.append(mk_wb(jq))
                if pend_wb and kb >= 4 * g + 1:
                    pend_wb.pop(0)()
            for op in fi:
                op()
            if final:
                while pend_wb:
                    pend_wb.pop(0)()
                return None
            return fst["xan"]

        # ---- stage T: transpose x_att back to [d, tok] for proj ----
        def stage_t_ops(w, xan):
            st = {}

            def mk_t(half):
                def op():
                    if half == 0:
                        st["ps"] = ps.tile([128, WQ], F32, tag="yx", bufs=2,
                                           name="xaT_ps")
                    for jq in range(2 * half, 2 * half + 2):
                        nc.tensor.transpose(
                            st["ps"][:, jq * 128:(jq + 1) * 128]
                            .bitcast(F32R),
                            xan[:, jq, :].bitcast(F32R),
                            ident_r[:].bitcast(F32R))
                return op

            def cp():
                nc.vector.tensor_copy(xaw[w][:], st["ps"][:])
            return [mk_t(0), mk_t(1), cp]

        # ---- stage C: proj partials + writeback for one token window ----
        def _copy(eng, out, in_):
            if eng == "scalar":
                nc.scalar.copy(out, in_)
            else:
                getattr(nc, eng).tensor_copy(out, in_)

        def stage_c_ops(w, tags=("yx",), pair=False, bufs=2,
                        engines=("vector", "vector", "vector", "scalar")):
            ops = []

            def mk_mm(i, nb, cc):
                def op():
                    st = ops_state.setdefault(nb, {})
                    if cc == 0:
                        st["y"] = y_pool.tile([128, D], BF16, tag="y",
                                              name="y_sb")
                        if pair:
                            st["yp"] = ps.tile([128, 2, 512], F32,
                                               tag=tags[i % len(tags)],
                                               bufs=1, name="yp")
                    if not pair:
                        st["yp%d" % cc] = ps.tile(
                            [128, 512], F32, tag=tags[(2 * i + cc)
                                                      % len(tags)],
                            bufs=bufs, name="yp")
                    yp_ap = (st["yp"][:, cc, :] if pair
                             else st["yp%d" % cc][:])
                    nc.tensor.matmul(
                        yp_ap,
                        xaw[nb // 4][:, (nb % 4) * 128:(nb % 4 + 1) * 128],
                        wp_sb[:, cc * 512:(cc + 1) * 512],
                        start=True, stop=True)
                    if pair and cc == 1:
                        _copy(engines[i % len(engines)], st["y"][:],
                              st["yp"][:])
                        nc.sync.dma_start(
                            out=y[nb * 128:(nb + 1) * 128, :], in_=st["y"][:])
                    elif not pair:
                        _copy(engines[(2 * i + cc) % len(engines)],
                              st["y"][:, cc * 512:(cc + 1) * 512], yp_ap)
                        if cc == 1:
                            nc.sync.dma_start(
                                out=y[nb * 128:(nb + 1) * 128, :],
                                in_=st["y"][:])
                return op

            ops_state = {}
            i = 0
            for nb in range(4 * w, 4 * w + 4):
                for cc in range(2):
                    ops.append(mk_mm(i, nb, cc))
                i += 1
            return ops

        # ---- software-pipelined emission ----
        mk_xw_dma(0, split=True)()
        for op in stage_a_ops(0):
            op()
        mk_xw_dma(1)()
        prev_xans = None
        for w in range(NW):
            final = (w == NW - 1)
            filler = []
            if prev_xans is not None:
                filler += stage_t_ops(w - 1, prev_xans)
            if w + 1 < NW:
                filler += stage_a_ops(w + 1)
            if prev_xans is not None:
                filler += stage_c_ops(w - 1, tags=("qk",) if final
                                      else ("yx",), pair=False,
                                      bufs=1 if final else 2)
            if w + 2 < NW:
                filler.append(mk_xw_dma(w + 2))
            prev_xans = stage_b(w, filler, final=final)


def _make_mask():
    p = np.arange(128)[:, None]
    j = np.arange(896)[None, :]
    return (j >= p + 384).astype(np.float32)


def _host_scales(W_qkv, u_qkv, sigma_qkv, W_proj, u_proj, sigma_proj):
    """Power-iteration spectral norm in fp32, exactly as the reference:
    v = normalize(W u); sigma = ||W^T v||."""
    def sig(W, u):
        v = (W @ u).astype(np.float32)
        v = v / np.float32(np.linalg.norm(v))
        u2 = (W.T @ v).astype(np.float32)
        return np.float32(np.linalg.norm(u2))
    c_qkv = np.float32(sigma_qkv[0]) / sig(W_qkv, u_qkv)
    c_proj = np.float32(sigma_proj[0]) / sig(W_proj, u_proj)
    return np.float32(c_qkv), np.float32(c_proj)


def make_in_maps(batch, W_qkv, u_qkv, sigma_qkv, W_proj, u_proj, sigma_proj):
    batch = np.asarray(batch, np.float32)
    W_qkv = np.asarray(W_qkv, np.float32)
    u_qkv = np.asarray(u_qkv, np.float32)
    sigma_qkv = np.asarray(sigma_qkv, np.float32)
    W_proj = np.asarray(W_proj, np.float32)
    u_proj = np.asarray(u_proj, np.float32)
    sigma_proj = np.asarray(sigma_proj, np.float32)
    bf16 = ml_dtypes.bfloat16
    x = batch.reshape(NTOK, D)
    xt = np.ascontiguousarray(x.T).astype(bf16)
    c_qkv, c_proj = _host_scales(W_qkv, u_qkv, sigma_qkv,
                                 W_proj, u_proj, sigma_proj)
    sqk = np.float32(c_qkv * HD ** -0.25)
    wq_all = (W_qkv[:, 0:D] * sqk).astype(bf16)
    wk_all = (W_qkv[:, D:2 * D] * sqk).astype(bf16)
    wv_all = (W_qkv[:, 2 * D:3 * D] * c_qkv).astype(bf16)
    wp_all = (W_proj * c_proj).astype(bf16)
    mask = _make_mask().astype(bf16)
    in_maps = []
    for c in range(N_CORES):
        cs = slice(128 * c, 128 * (c + 1))
        in_maps.append({
            "xt": xt,
            "wqkv": np.ascontiguousarray(
                np.concatenate([wq_all[:, cs], wk_all[:, cs],
                                wv_all[:, cs]], axis=1)),
            "wp": np.ascontiguousarray(wp_all[cs, :]),
            "mask": mask,
        })
    return in_maps


_NC_CACHE = None


def build_nc():
    global _NC_CACHE
    if _NC_CACHE is None:
        nc = bass.Bass("TRN2", target_bir_lowering=False, debug=False,
                       num_devices=N_CORES)
        with _TileContextSplit(nc) as tc:
            _build_body(nc, tc)
        _NC_CACHE = nc
    return _NC_CACHE


def kernel(batch, W_qkv, u_qkv, sigma_qkv, W_proj, u_proj, sigma_proj):
    in_maps = make_in_maps(batch, W_qkv, u_qkv, sigma_qkv,
                           W_proj, u_proj, sigma_proj)
    nc = build_nc()
    res = run_bass_kernel_spmd(nc, in_maps, list(range(N_CORES)))
    y = np.zeros((NTOK, D), np.float64)
    for c in range(N_CORES):
        y += res.results[c]["y"].astype(np.float64)
    return y.astype(np.float32).reshape(BATCH, NSEQ, D)
